# revision 13
# baseline (speedup 1.0000x reference)
"""Trainium2 Bass kernel for nn_Attention_72404558676364.

Math: the reference computes
    pre[l,b,:] = hs_encoder[l,b,:] @ We.T + (hidden @ Wh.T + b_att)[b,:]
    attn[b,l]  = pre[l,b,:] . v
    out        = softmax(attn, axis=l)
Softmax over l is shift-invariant, so the hidden/Wh/b_att term (constant in
l for fixed b) cancels exactly and the einsum collapses to a single matvec:
    attn[b,l] = hs_encoder[l,b,:] . w_eff,   w_eff = We.T @ v
The device does one pass over hs_encoder plus the small We.T @ v.

Sharding: data-parallel over batch; core c handles batches [8c, 8c+8).
hs_encoder shards are pre-transposed on the host to [H, Bc*L] so every DMA is
contiguous per partition (fp32 cannot use the DMA-transpose xbar and
strided-AP transposes are ~19x slower).

Precision/speed: fp32 matmul runs at 1/4 PE rate.  Instead, every matmul
operand is split into an fp16 (hi, lo) pair — hs and We on the host, w_eff on
the device — and each dot product is computed with 3-term compensation
    x.y ~= hi(x).hi(y) + lo(x).hi(y) + hi(x).lo(y)
which runs at full 1-cycle/column PE rate with ~1e-7 relative element error, and the
byte volume shipped over HBM is identical to fp32.
"""

import numpy as np

import concourse.bass as bass
import concourse.mybir as mybir
import concourse.tile as tile
from concourse.bass_utils import run_bass_kernel_spmd

H = 1024
L = 512
B = 64
NCORES = 8
BC = B // NCORES  # batches per core
P = 128
HC = H // P  # 128-wide chunks of the contraction dim

F32 = mybir.dt.float32
F16 = mybir.dt.float16
F16_NP = np.float16

_split_n = 0


def _split_multi_waits(nc):
    """Hoist extra sem waits onto same-engine NOPs.

    The walrus build in this container rejects any instruction carrying more
    than one sync-wait ("Too many sync wait commands"), but Tile emits
    multi-wait instructions whenever one op depends on several producers.
    A NOP on the same engine immediately before the instruction waits
    equivalently (per-engine program order).
    """
    global _split_n
    for fn in nc.m.functions:
        for blk in fn.blocks:
            new_insts = []
            for inst in blk.instructions:
                si = getattr(inst, "sync_info", None)
                if si is not None and si.on_wait and len(si.on_wait) > 1:
                    waits = list(si.on_wait)
                    si.on_wait = waits[:1]
                    for w in waits[1:]:
                        _split_n += 1
                        new_insts.append(
                            mybir.InstNoOp(
                                name=f"I-wsplit-{_split_n}",
                                engine=inst.engine,
                                sync_info=mybir.SyncInfo(
                                    on_wait=[w], on_update=[]
                                ),
                                bass_nofuse=True,
                            )
                        )
                new_insts.append(inst)
            blk.instructions = new_insts


def _build():
    nc = bass.Bass(target_bir_lowering=False, enable_partition_id=False)
    hs_hi = nc.dram_tensor("hs_hi", [H, BC * L], F16, kind="ExternalInput")
    hs_lo = nc.dram_tensor("hs_lo", [H, BC * L], F16, kind="ExternalInput")
    we_hi = nc.dram_tensor("We_hi", [H, H], F16, kind="ExternalInput")
    we_lo = nc.dram_tensor("We_lo", [H, H], F16, kind="ExternalInput")
    v_hi = nc.dram_tensor("v_hi", [P, HC], F16, kind="ExternalInput")
    v_lo = nc.dram_tensor("v_lo", [P, HC], F16, kind="ExternalInput")
    out = nc.dram_tensor("out", [BC, L], F32, kind="ExternalOutput")

    with tile.TileContext(nc) as tc:
        with (
            tc.tile_pool(name="singles", bufs=1) as singles,
            tc.tile_pool(name="hsh", bufs=16) as hsh_pool,
            tc.tile_pool(name="hsl", bufs=16) as hsl_pool,
            tc.tile_pool(name="srow", bufs=2) as srow_pool,
            tc.tile_pool(name="psw", bufs=2, space="PSUM") as psw_pool,
            tc.tile_pool(name="pst", bufs=2, space="PSUM") as pst_pool,
            tc.tile_pool(name="pss", bufs=4, space="PSUM") as pss_pool,
        ):
            # ---- small operands ---------------------------------------
            vh_sb = singles.tile([P, HC], F16)
            nc.sync.dma_start(out=vh_sb[:], in_=v_hi[:])
            vl_sb = singles.tile([P, HC], F16)
            nc.sync.dma_start(out=vl_sb[:], in_=v_lo[:])
            ident = singles.tile([1, 1], F32)
            nc.vector.memset(ident[:], 1.0)

            # Per-chunk We DMAs so the first flip matmul starts after ~1 us,
            # split across both HWDGE rings (SP and ACT).
            weh_sb = singles.tile([P, HC, H], F16)
            wel_sb = singles.tile([P, HC, H], F16)
            for hc in range(HC):
                nc.sync.dma_start(
                    out=weh_sb[:, hc, :], in_=we_hi[hc * P : (hc + 1) * P, :]
                )
                nc.scalar.dma_start(
                    out=wel_sb[:, hc, :], in_=we_lo[hc * P : (hc + 1) * P, :]
                )

            # ---- w_eff = We.T @ v as a [1, H] fp32 row ----------------
            # lhsT = v chunk [128,1]; rhs = We chunk [128, 512]; 3-term
            # bf16 compensation accumulates into one psum per k-half.
            w_row = singles.tile([1, H], F32)
            for half in range(2):
                ph = psw_pool.tile([1, L], F32)
                n_mm = 3 * HC
                i_mm = 0
                for hc in range(HC):
                    ksl = slice(half * L, (half + 1) * L)
                    for lhs, rhs in (
                        (vh_sb, weh_sb),
                        (vl_sb, weh_sb),
                        (vh_sb, wel_sb),
                    ):
                        nc.tensor.matmul(
                            ph[:],
                            lhsT=lhs[:, hc : hc + 1],
                            rhs=rhs[:, hc, ksl],
                            start=(i_mm == 0),
                            stop=(i_mm == n_mm - 1),
                        )
                        i_mm += 1
                nc.scalar.copy(out=w_row[0:1, half * L : (half + 1) * L], in_=ph[:])

            # ---- w_row -> w_cols[p, hc] = w_eff[hc*128+p], bf16 pair --
            w_cols = singles.tile([P, HC], F32)
            for hc in range(HC):
                pt = pst_pool.tile([P, 1], F32)
                nc.tensor.transpose(
                    pt[:], w_row[0:1, hc * P : (hc + 1) * P], ident[:]
                )
                nc.vector.tensor_copy(out=w_cols[:, hc : hc + 1], in_=pt[:])
            wh_cols = singles.tile([P, HC], F16)
            nc.vector.tensor_copy(out=wh_cols[:], in_=w_cols[:])
            wh_f32 = singles.tile([P, HC], F32)
            nc.vector.tensor_copy(out=wh_f32[:], in_=wh_cols[:])
            wl_f32 = singles.tile([P, HC], F32)
            nc.vector.tensor_sub(out=wl_f32[:], in0=w_cols[:], in1=wh_f32[:])
            wl_cols = singles.tile([P, HC], F16)
            nc.vector.tensor_copy(out=wl_cols[:], in_=wl_f32[:])

            # ---- scores[j, l] = hsT[:, j*L+l] . w_eff ------------------
            # hs tiles [128, 2048] span four batches; hi via the SP HWDGE
            # ring, lo via the ACT ring, all issued early.
            scoresb = singles.tile([BC, L], F32)
            for jp in range(BC // 4):
                hi_tiles, lo_tiles = [], []
                for hc in range(HC):
                    th = hsh_pool.tile([P, 4 * L], F16)
                    nc.sync.dma_start(
                        out=th[:],
                        in_=hs_hi[
                            hc * P : (hc + 1) * P, jp * 4 * L : (jp + 1) * 4 * L
                        ],
                    )
                    hi_tiles.append(th)
                    tl = hsl_pool.tile([P, 4 * L], F16)
                    nc.scalar.dma_start(
                        out=tl[:],
                        in_=hs_lo[
                            hc * P : (hc + 1) * P, jp * 4 * L : (jp + 1) * 4 * L
                        ],
                    )
                    lo_tiles.append(tl)
                for j in range(4 * jp, 4 * jp + 4):
                    off = (j % 4) * L
                    ps = pss_pool.tile([1, L], F32)
                    n_mm = 3 * HC
                    i_mm = 0
                    for hc in range(HC):
                        for lhs, rhs in (
                            (wh_cols, hi_tiles[hc]),
                            (wl_cols, hi_tiles[hc]),
                            (wh_cols, lo_tiles[hc]),
                        ):
                            nc.tensor.matmul(
                                ps[:],
                                lhsT=lhs[:, hc : hc + 1],
                                rhs=rhs[:, off : off + L],
                                start=(i_mm == 0),
                                stop=(i_mm == n_mm - 1),
                            )
                            i_mm += 1
                    srow = srow_pool.tile([1, L], F32)
                    nc.scalar.copy(out=srow[:], in_=ps[:])
                    # SWDGE keeps this off the HWDGE rings so its wait on the
                    # ACT copy never stalls the streaming input DMAs.
                    nc.gpsimd.dma_start(out=scoresb[j : j + 1, :], in_=srow[:])

            # ---- softmax over l per batch row --------------------------
            negmax = singles.tile([BC, 1], F32)
            nc.vector.reduce_max(
                out=negmax[:], in_=scoresb[:], axis=mybir.AxisListType.X,
                negate=True,
            )
            exps = singles.tile([BC, L], F32)
            sums = singles.tile([BC, 1], F32)
            nc.scalar.activation(
                out=exps[:],
                in_=scoresb[:],
                func=mybir.ActivationFunctionType.Exp,
                bias=negmax[:],
                scale=1.0,
                accum_out=sums[:],
            )
            rsum = singles.tile([BC, 1], F32)
            nc.vector.reciprocal(out=rsum[:], in_=sums[:])
            outb = singles.tile([BC, L], F32)
            nc.vector.tensor_scalar_mul(out=outb[:], in0=exps[:], scalar1=rsum[:])
            nc.gpsimd.dma_start(out=out[:], in_=outb[:])

    _split_multi_waits(nc)
    return nc


def _hi_lo(a):
    hi = a.astype(F16_NP)
    lo = (a - hi.astype(np.float32)).astype(F16_NP)
    return hi, lo


_NC_CACHE = None


def _make_in_maps(hs_encoder, W_att, vector):
    hs_encoder = np.ascontiguousarray(hs_encoder, dtype=np.float32)
    we_hi, we_lo = _hi_lo(np.ascontiguousarray(W_att[:, H:], dtype=np.float32))
    v_arr = np.ascontiguousarray(
        np.asarray(vector, dtype=np.float32)[:, 0].reshape(HC, P).T
    )
    v_hi, v_lo = _hi_lo(v_arr)

    in_maps = []
    for c in range(NCORES):
        shard = hs_encoder[:, c * BC : (c + 1) * BC, :]  # [L, BC, H]
        hst = np.ascontiguousarray(shard.transpose(2, 1, 0).reshape(H, BC * L))
        hs_hi, hs_lo = _hi_lo(hst)
        in_maps.append(
            {
                "hs_hi": hs_hi, "hs_lo": hs_lo,
                "We_hi": we_hi, "We_lo": we_lo,
                "v_hi": v_hi, "v_lo": v_lo,
            }
        )
    return in_maps


def kernel(hidden, hs_encoder, W_att, b_att, vector):
    global _NC_CACHE
    if _NC_CACHE is None:
        _NC_CACHE = _build()
    nc = _NC_CACHE

    in_maps = _make_in_maps(hs_encoder, W_att, vector)
    res = run_bass_kernel_spmd(nc, in_maps, core_ids=list(range(NCORES)))
    out = np.concatenate([res.results[c]["out"] for c in range(NCORES)], axis=0)
    return out[:, None, :].astype(np.float32)


# revision 15
# speedup vs baseline: 1.1528x; 1.1528x over previous
"""Trainium2 Bass kernel for nn_Attention_72404558676364.

Math: the reference computes
    pre[l,b,:] = hs_encoder[l,b,:] @ We.T + (hidden @ Wh.T + b_att)[b,:]
    attn[b,l]  = pre[l,b,:] . v
    out        = softmax(attn, axis=l)
Softmax over l is shift-invariant, so the hidden/Wh/b_att term (constant in
l for fixed b) cancels exactly and the einsum collapses to a single matvec:
    attn[b,l] = hs_encoder[l,b,:] . w_eff,   w_eff = We.T @ v
The device does one pass over hs_encoder plus the small We.T @ v.

Sharding: data-parallel over batch; core c handles batches [8c, 8c+8).
hs_encoder shards are pre-transposed on the host to [H, Bc*L] so every DMA is
contiguous per partition (fp32 cannot use the DMA-transpose xbar and
strided-AP transposes are ~19x slower).

Precision/speed: fp32 matmul runs at 1/4 PE rate.  Instead, every matmul
operand is split into an fp16 (hi, lo) pair — hs and We on the host, w_eff on
the device — and each dot product is computed with 3-term compensation
    x.y ~= hi(x).hi(y) + lo(x).hi(y) + hi(x).lo(y)
which runs at full 1-cycle/column PE rate with ~1e-7 relative element error, and the
byte volume shipped over HBM is identical to fp32.
"""

import numpy as np

import concourse.bass as bass
import concourse.mybir as mybir
import concourse.tile as tile
from concourse.bass_utils import run_bass_kernel_spmd

H = 1024
L = 512
B = 64
NCORES = 8
BC = B // NCORES  # batches per core
P = 128
HC = H // P  # 128-wide chunks of the contraction dim

F32 = mybir.dt.float32
F16 = mybir.dt.float16
F16_NP = np.float16

_split_n = 0


def _split_multi_waits(nc):
    """Hoist extra sem waits onto same-engine NOPs.

    The walrus build in this container rejects any instruction carrying more
    than one sync-wait ("Too many sync wait commands"), but Tile emits
    multi-wait instructions whenever one op depends on several producers.
    A NOP on the same engine immediately before the instruction waits
    equivalently (per-engine program order).
    """
    global _split_n
    for fn in nc.m.functions:
        for blk in fn.blocks:
            new_insts = []
            for inst in blk.instructions:
                si = getattr(inst, "sync_info", None)
                if si is not None and si.on_wait and len(si.on_wait) > 1:
                    waits = list(si.on_wait)
                    si.on_wait = waits[:1]
                    for w in waits[1:]:
                        _split_n += 1
                        new_insts.append(
                            mybir.InstNoOp(
                                name=f"I-wsplit-{_split_n}",
                                engine=inst.engine,
                                sync_info=mybir.SyncInfo(
                                    on_wait=[w], on_update=[]
                                ),
                                bass_nofuse=True,
                            )
                        )
                new_insts.append(inst)
            blk.instructions = new_insts


def _build():
    nc = bass.Bass(target_bir_lowering=False, enable_partition_id=False)
    hs_hi = nc.dram_tensor("hs_hi", [H, BC * L], F16, kind="ExternalInput")
    hs_lo = nc.dram_tensor("hs_lo", [H, BC * L], F16, kind="ExternalInput")
    we_hi = nc.dram_tensor("We_hi", [H, H], F16, kind="ExternalInput")
    we_lo = nc.dram_tensor("We_lo", [H, H], F16, kind="ExternalInput")
    v_hi = nc.dram_tensor("v_hi", [P, HC], F16, kind="ExternalInput")
    v_lo = nc.dram_tensor("v_lo", [P, HC], F16, kind="ExternalInput")
    out = nc.dram_tensor("out", [BC, L], F32, kind="ExternalOutput")

    with tile.TileContext(nc) as tc:
        with (
            tc.tile_pool(name="singles", bufs=1) as singles,
            tc.tile_pool(name="hsh", bufs=16) as hsh_pool,
            tc.tile_pool(name="hsl", bufs=16) as hsl_pool,
            tc.tile_pool(name="srow", bufs=2) as srow_pool,
            tc.tile_pool(name="psw", bufs=2, space="PSUM") as psw_pool,
            tc.tile_pool(name="pst", bufs=2, space="PSUM") as pst_pool,
            tc.tile_pool(name="pss", bufs=4, space="PSUM") as pss_pool,
        ):
            # ---- small operands ---------------------------------------
            vh_sb = singles.tile([P, HC], F16)
            nc.sync.dma_start(out=vh_sb[:], in_=v_hi[:])
            vl_sb = singles.tile([P, HC], F16)
            nc.sync.dma_start(out=vl_sb[:], in_=v_lo[:])
            ident = singles.tile([1, 1], F32)
            nc.vector.memset(ident[:], 1.0)

            # Per-chunk We DMAs so the first flip matmul starts after ~1 us,
            # split across both HWDGE rings (SP and ACT).
            weh_sb = singles.tile([P, HC, H], F16)
            wel_sb = singles.tile([P, HC, H], F16)
            for hc in range(HC):
                nc.sync.dma_start(
                    out=weh_sb[:, hc, :], in_=we_hi[hc * P : (hc + 1) * P, :]
                )
                nc.scalar.dma_start(
                    out=wel_sb[:, hc, :], in_=we_lo[hc * P : (hc + 1) * P, :]
                )

            # ---- w_eff = We.T @ v as a [1, H] fp32 row ----------------
            # lhsT = v chunk [128,1]; rhs = We chunk [128, 512]; 3-term
            # bf16 compensation accumulates into one psum per k-half.
            w_row = singles.tile([1, H], F32)
            for half in range(2):
                ph = psw_pool.tile([1, L], F32)
                n_mm = 3 * HC
                i_mm = 0
                for hc in range(HC):
                    ksl = slice(half * L, (half + 1) * L)
                    for lhs, rhs in (
                        (vh_sb, weh_sb),
                        (vl_sb, weh_sb),
                        (vh_sb, wel_sb),
                    ):
                        nc.tensor.matmul(
                            ph[:],
                            lhsT=lhs[:, hc : hc + 1],
                            rhs=rhs[:, hc, ksl],
                            start=(i_mm == 0),
                            stop=(i_mm == n_mm - 1),
                        )
                        i_mm += 1
                nc.scalar.copy(out=w_row[0:1, half * L : (half + 1) * L], in_=ph[:])

            # ---- w_row -> w_cols[p, hc] = w_eff[hc*128+p], bf16 pair --
            w_cols = singles.tile([P, HC], F32)
            for hc in range(HC):
                pt = pst_pool.tile([P, 1], F32)
                nc.tensor.transpose(
                    pt[:], w_row[0:1, hc * P : (hc + 1) * P], ident[:]
                )
                nc.vector.tensor_copy(out=w_cols[:, hc : hc + 1], in_=pt[:])
            wh_cols = singles.tile([P, HC], F16)
            nc.vector.tensor_copy(out=wh_cols[:], in_=w_cols[:])
            wh_f32 = singles.tile([P, HC], F32)
            nc.vector.tensor_copy(out=wh_f32[:], in_=wh_cols[:])
            wl_f32 = singles.tile([P, HC], F32)
            nc.vector.tensor_sub(out=wl_f32[:], in0=w_cols[:], in1=wh_f32[:])
            wl_cols = singles.tile([P, HC], F16)
            nc.vector.tensor_copy(out=wl_cols[:], in_=wl_f32[:])

            # ---- scores[j, l] = hsT[:, j*L+l] . w_eff ------------------
            # hs tiles [128, 2048] span four batches; hi via the SP HWDGE
            # ring, lo via the ACT ring, all issued early.
            for jp in range(BC // 4):
                hi_tiles, lo_tiles = [], []
                for hc in range(HC):
                    th = hsh_pool.tile([P, 4 * L], F16)
                    nc.sync.dma_start(
                        out=th[:],
                        in_=hs_hi[
                            hc * P : (hc + 1) * P, jp * 4 * L : (jp + 1) * 4 * L
                        ],
                    )
                    hi_tiles.append(th)
                    tl = hsl_pool.tile([P, 4 * L], F16)
                    nc.scalar.dma_start(
                        out=tl[:],
                        in_=hs_lo[
                            hc * P : (hc + 1) * P, jp * 4 * L : (jp + 1) * 4 * L
                        ],
                    )
                    lo_tiles.append(tl)
                for j in range(4 * jp, 4 * jp + 4):
                    off = (j % 4) * L
                    ps = pss_pool.tile([1, L], F32)
                    n_mm = 3 * HC
                    i_mm = 0
                    for hc in range(HC):
                        for lhs, rhs in (
                            (wh_cols, hi_tiles[hc]),
                            (wl_cols, hi_tiles[hc]),
                            (wh_cols, lo_tiles[hc]),
                        ):
                            nc.tensor.matmul(
                                ps[:],
                                lhsT=lhs[:, hc : hc + 1],
                                rhs=rhs[:, off : off + L],
                                start=(i_mm == 0),
                                stop=(i_mm == n_mm - 1),
                            )
                            i_mm += 1
                    # Per-batch softmax on idle DVE/ACT while later batches'
                    # matmuls stream; only the last batch's chain is exposed.
                    srow = srow_pool.tile([1, L], F32)
                    nc.scalar.copy(out=srow[:], in_=ps[:])
                    negmax = srow_pool.tile([1, 1], F32)
                    nc.vector.reduce_max(
                        out=negmax[:], in_=srow[:], axis=mybir.AxisListType.X,
                        negate=True,
                    )
                    exps = srow_pool.tile([1, L], F32)
                    sums = srow_pool.tile([1, 1], F32)
                    nc.scalar.activation(
                        out=exps[:],
                        in_=srow[:],
                        func=mybir.ActivationFunctionType.Exp,
                        bias=negmax[:],
                        scale=1.0,
                        accum_out=sums[:],
                    )
                    rsum = srow_pool.tile([1, 1], F32)
                    nc.vector.reciprocal(out=rsum[:], in_=sums[:])
                    orow = srow_pool.tile([1, L], F32)
                    nc.vector.tensor_scalar_mul(
                        out=orow[:], in0=exps[:], scalar1=rsum[:]
                    )
                    # SWDGE keeps this off the HWDGE rings so its waits never
                    # stall the streaming input DMAs.
                    nc.gpsimd.dma_start(out=out[j : j + 1, :], in_=orow[:])

    _split_multi_waits(nc)
    return nc


def _hi_lo(a):
    hi = a.astype(F16_NP)
    lo = (a - hi.astype(np.float32)).astype(F16_NP)
    return hi, lo


_NC_CACHE = None


def _make_in_maps(hs_encoder, W_att, vector):
    hs_encoder = np.ascontiguousarray(hs_encoder, dtype=np.float32)
    we_hi, we_lo = _hi_lo(np.ascontiguousarray(W_att[:, H:], dtype=np.float32))
    v_arr = np.ascontiguousarray(
        np.asarray(vector, dtype=np.float32)[:, 0].reshape(HC, P).T
    )
    v_hi, v_lo = _hi_lo(v_arr)

    in_maps = []
    for c in range(NCORES):
        shard = hs_encoder[:, c * BC : (c + 1) * BC, :]  # [L, BC, H]
        hst = np.ascontiguousarray(shard.transpose(2, 1, 0).reshape(H, BC * L))
        hs_hi, hs_lo = _hi_lo(hst)
        in_maps.append(
            {
                "hs_hi": hs_hi, "hs_lo": hs_lo,
                "We_hi": we_hi, "We_lo": we_lo,
                "v_hi": v_hi, "v_lo": v_lo,
            }
        )
    return in_maps


def kernel(hidden, hs_encoder, W_att, b_att, vector):
    global _NC_CACHE
    if _NC_CACHE is None:
        _NC_CACHE = _build()
    nc = _NC_CACHE

    in_maps = _make_in_maps(hs_encoder, W_att, vector)
    res = run_bass_kernel_spmd(nc, in_maps, core_ids=list(range(NCORES)))
    out = np.concatenate([res.results[c]["out"] for c in range(NCORES)], axis=0)
    return out[:, None, :].astype(np.float32)
